# revision 54
# baseline (speedup 1.0000x reference)
"""DCGRU cell on 8 Trainium2 NeuronCores.

Strategy (dst-sharded graph partitioning, overlapped AllGather):
  - Nodes are sharded into 8 contiguous ranges (one per core). Edges are
    assigned to the core owning dst; within a core, dst nodes are processed
    in blocks of 128 grouped into super-blocks of SB_BLOCKS for gathering.
  - Pass 1 gathers x1 = [feat, state] rows (bf16, 256B) per edge and
    segment-sums them via one-hot matmuls into PSUM (transposed layout).
    zr/r^T/y2 are produced without PE transposes: the host supplies feat^T
    and state^T so y2 = feat @ Wc_top + (r*state) @ Wc_bot uses featT/rsT
    directly as lhsT.
  - y2 (bf16) is AllGathered in KC block-range chunks, each issued as soon
    as its blocks finish in pass 1, so the collective overlaps phase-A
    compute (collective cost model: 15us fixed + bytes/40GBps).
  - Pass 2 gathers y2 rows (128B) directly, sweeping source-chunks k-outer
    so sweep k only waits on collective chunk k; per (block, sweep) partial
    PSUM aggregates accumulate into an SBUF accumulator.
"""

import numpy as np

import concourse.bass as bass
import concourse.bacc as bacc
import concourse.mybir as mybir
import concourse.tile as tile
from concourse.bass_utils import run_bass_kernel_spmd
from concourse.library_config import mlp
from concourse.masks import make_identity

N_NODES = 50000
N_EDGES = 640000
HID = 64
N_CORES = 8
BLK = 128           # dst nodes per block (= PSUM partition dim)
SB_BLOCKS = 4       # dst blocks per super-block (PSUM live tiles = SB_BLOCKS)
MAX_G_CHUNK = 32    # cap on groups per dma_gather instruction
KC = 4              # source chunks for the AllGather / pass-2 sweeps

F32 = mybir.dt.float32
BF16 = mybir.dt.bfloat16
I16 = mybir.dt.int16


def _kchunk_blocks(nblk):
    """Block ranges per source chunk: decreasing sizes so each sweep's
    collective chunk lands just before the sweep needs it."""
    if nblk == 49 and KC == 4:
        sizes = [4, 22, 17, 6]
    else:
        base = nblk // KC
        rem = nblk % KC
        sizes = [base + (1 if i < rem else 0) for i in range(KC)]
    bounds = np.concatenate([[0], np.cumsum(sizes)])
    return bounds  # len KC+1, bounds[-1] == nblk


def _build_tables(e_owner, e_ukey, e_idxval, e_dloc, e_w, unit_off, n_units):
    """Slot edges into (group, lane) and build idx16/dst/w tables.

    e_* are per-edge arrays; e_ukey is the unit id (dense, 0..n_units-1);
    e_idxval is the int16 gather index value. Returns (idx16, dst_t, w_t).
    """
    e = len(e_owner)
    ngroups = int(unit_off[-1])
    ck = e_owner * n_units + e_ukey
    order = np.argsort(ck, kind="stable")
    ck_s = ck[order]
    owner_s = e_owner[order]
    idxval_s = e_idxval[order]
    ukey_s = e_ukey[order]
    dloc_s = e_dloc[order]
    w_s = e_w[order]
    bucket_start = np.searchsorted(ck_s, np.arange(N_CORES * n_units))
    rank = np.arange(e) - bucket_start[ck_s]
    g_global = unit_off[ukey_s] + rank // BLK
    lane = rank % BLK

    idx16 = np.zeros((N_CORES, 16, 8 * ngroups), np.int16)
    dst_t = np.zeros((N_CORES, BLK, ngroups), np.float32)
    w_t = np.zeros((N_CORES, BLK, ngroups), np.float32)
    idx16[owner_s, lane % 16, 8 * g_global + lane // 16] = idxval_s.astype(np.int16)
    dst_t[owner_s, lane, g_global] = dloc_s
    w_t[owner_s, lane, g_global] = w_s
    idx16 = np.tile(idx16, (1, 8, 1))
    return idx16, dst_t, w_t


def _prep_edges(dst, src, edge_weight, n_nodes, n_cores):
    """Partition edges by dst core/block; build pass-1 and pass-2 tables."""
    shard = n_nodes // n_cores
    nblk = (shard + BLK - 1) // BLK
    split = n_nodes // 2
    e = len(dst)

    dsts = dst.astype(np.int64)
    srcs = src.astype(np.int64)
    owner = dsts // shard
    local = dsts - owner * shard

    # Balance in-degree across blocks: per core, deal nodes (sorted by
    # in-degree, desc) round-robin over blocks. pos[core, orig_local] is the
    # node's new row; node tables / shard rows / outputs use this order.
    deg = np.zeros(n_nodes, np.int64)
    np.add.at(deg, dsts, 1)
    pos = np.empty((n_cores, shard), np.int64)
    blk_fill = np.empty(nblk, np.int64)
    cap = np.full(nblk, BLK, np.int64)
    cap[nblk - 1] = shard - (nblk - 1) * BLK
    for p in range(n_cores):
        nodes = np.argsort(-deg[p * shard : (p + 1) * shard], kind="stable")
        blk_fill[:] = 0
        bi = 0
        for n in nodes:
            while blk_fill[bi % nblk] >= cap[bi % nblk]:
                bi += 1
            b = bi % nblk
            pos[p, n] = b * BLK + blk_fill[b]
            blk_fill[b] += 1
            bi += 1
    newloc = pos[owner, local]
    b_of = newloc // BLK
    dloc = (newloc % BLK).astype(np.float32)
    w_f = edge_weight.astype(np.float32)

    # permuted global src position
    src_pos = pos[srcs // shard, srcs % shard] + (srcs // shard) * shard

    # ---------------- pass 1: units = (super-block, class, block) ----------
    cls = (src_pos >= split).astype(np.int64)
    src_local1 = src_pos - cls * split

    cnt = np.zeros((n_cores, nblk, 2), np.int64)
    np.add.at(cnt, (owner, b_of, cls), 1)
    gpbc = -(-cnt.max(axis=0) // BLK)  # [nblk, 2]
    empty = gpbc.sum(axis=1) == 0
    gpbc[empty, 0] = 1

    unit_order = []
    for sb0 in range(0, nblk, SB_BLOCKS):
        sbb = range(sb0, min(sb0 + SB_BLOCKS, nblk))
        for c in range(2):
            for b in sbb:
                unit_order.append((b, c))
    unit_sizes = np.array([gpbc[b, c] for b, c in unit_order], np.int64)
    unit_off = np.concatenate([[0], np.cumsum(unit_sizes)])
    ngroups = int(unit_off[-1])
    unit_idx = {bc: i for i, bc in enumerate(unit_order)}

    blk_of_g = np.zeros(ngroups, np.int64)
    for i, (b, c) in enumerate(unit_order):
        blk_of_g[unit_off[i] : unit_off[i + 1]] = b
    first_g = np.full(nblk, -1, np.int64)
    last_g = np.full(nblk, -1, np.int64)
    for g in range(ngroups):
        b = blk_of_g[g]
        if first_g[b] < 0:
            first_g[b] = g
        last_g[b] = g

    # gather chunks: contiguous same-class unit runs within a super-block
    chunks = []
    i = 0
    while i < len(unit_order):
        c = unit_order[i][1]
        sb = unit_order[i][0] // SB_BLOCKS
        j = i
        while (
            j < len(unit_order)
            and unit_order[j][1] == c
            and unit_order[j][0] // SB_BLOCKS == sb
        ):
            j += 1
        g0, g1 = int(unit_off[i]), int(unit_off[j])
        for s in range(g0, g1, MAX_G_CHUNK):
            if s < g1:
                chunks.append((s, min(s + MAX_G_CHUNK, g1), c))
        i = j

    ukey1 = np.array([unit_idx[(b, c)] for b, c in zip(b_of, cls)], np.int64)
    idx16, dst_t, w_t = _build_tables(
        owner, ukey1, src_local1, dloc, w_f, unit_off, len(unit_order)
    )

    # ---------------- pass 2: units = (kchunk, block) ----------------------
    kb = _kchunk_blocks(nblk)  # block bounds, len KC+1
    krow = kb * BLK  # row bounds within shard (last may exceed shard)
    krow[-1] = shard
    rows_k = np.diff(krow)  # rows per chunk per core

    assert all(r % 2 == 0 for r in rows_k), "pair trick needs even chunk rows"
    off_k = np.concatenate([[0], np.cumsum(rows_k)])  # per-core chunk offsets
    sp_core = src_pos // shard
    sp_local = src_pos - sp_core * shard
    sp_blk = sp_local // BLK
    kc_of = np.searchsorted(kb[1:], sp_blk, side="right")
    # row in the chunk-major y2f_all tensor
    crow = 8 * off_k[kc_of] + sp_core * rows_k[kc_of] + (sp_local - krow[kc_of])
    idxval2 = crow // 2
    par2 = (crow % 2).astype(np.float32)

    # groups per dst block (edges k-sorted within each (core, block))
    cnt2 = np.zeros((n_cores, nblk), np.int64)
    np.add.at(cnt2, (owner, b_of), 1)
    gpb = np.maximum(-(-cnt2.max(axis=0) // BLK), 1)  # [nblk]

    # per-core rank within (block), with edges sorted by kc inside the block
    ck2 = (owner * nblk + b_of) * (KC + 1) + kc_of
    order2 = np.argsort(ck2, kind="stable")
    cb_key = (owner * nblk + b_of)[order2]
    kc_s = kc_of[order2]
    bucket2 = np.searchsorted(cb_key, np.arange(n_cores * nblk))
    rank2 = np.arange(e) - bucket2[cb_key]
    j_of = rank2 // BLK  # within-block group index

    # class of within-block group j = max over cores of kc at that rank range
    cls_jb = np.zeros((nblk, int(gpb.max())), np.int64)
    np.maximum.at(cls_jb, ((b_of[order2]), j_of), kc_s)

    # global group order: (class, block, j)
    tuples = []
    for b in range(nblk):
        for j in range(int(gpb[b])):
            tuples.append((int(cls_jb[b, j]), b, j))
    tuples.sort()
    ngroups2 = len(tuples)
    g_of = np.zeros((nblk, int(gpb.max())), np.int64)
    blk_of_g2 = np.zeros(ngroups2, np.int64)
    cls_of_g2 = np.zeros(ngroups2, np.int64)
    for g, (c, b, j) in enumerate(tuples):
        g_of[b, j] = g
        blk_of_g2[g] = b
        cls_of_g2[g] = c

    # per-block segments (contiguous same-class group runs in global order)
    seg_first = np.zeros(ngroups2, bool)
    seg_last = np.zeros(ngroups2, bool)
    seg_idx_g = np.zeros(ngroups2, np.int64)
    n_segs = np.zeros(nblk, np.int64)
    for b in range(nblk):
        gs = sorted(g_of[b, : int(gpb[b])])
        si = 0
        for i, g in enumerate(gs):
            if i == 0 or cls_of_g2[g] != cls_of_g2[gs[i - 1]]:
                seg_first[g] = True
                if i > 0:
                    seg_last[gs[i - 1]] = True
                    si += 1
            seg_idx_g[g] = si
        seg_last[gs[-1]] = True
        n_segs[b] = si + 1

    # gather chunks: same-class runs capped at MAX_G_CHUNK
    chunks2 = []
    i = 0
    while i < ngroups2:
        c = int(cls_of_g2[i])
        j = i
        while j < ngroups2 and cls_of_g2[j] == c:
            j += 1
        for s in range(i, j, MAX_G_CHUNK):
            chunks2.append((s, min(s + MAX_G_CHUNK, j), c))
        i = j

    # build tables with per-edge global group/lane
    owner_s2 = owner[order2]
    g_global2 = g_of[b_of[order2], j_of]
    lane2 = rank2 % BLK
    idxval2_s = idxval2[order2]
    dstp2_s = (dloc + BLK * par2)[order2]
    w2_s = w_f[order2]
    idx16c = np.zeros((N_CORES, 16, 8 * ngroups2), np.int16)
    dst2_t = np.zeros((N_CORES, BLK, ngroups2), np.float32)
    w2_t = np.zeros((N_CORES, BLK, ngroups2), np.float32)
    idx16c[owner_s2, lane2 % 16, 8 * g_global2 + lane2 // 16] = idxval2_s.astype(
        np.int16
    )
    dst2_t[owner_s2, lane2, g_global2] = dstp2_s
    w2_t[owner_s2, lane2, g_global2] = w2_s
    idx16c = np.tile(idx16c, (1, 8, 1))

    plan = {
        "chunks": chunks,
        "chunks2": chunks2,
        "blk_of_g": [int(x) for x in blk_of_g],
        "first_g": [int(x) for x in first_g],
        "last_g": [int(x) for x in last_g],
        "blk_of_g2": [int(x) for x in blk_of_g2],
        "seg_first": seg_first,
        "seg_last": seg_last,
        "seg_idx_g": seg_idx_g,
        "n_segs": n_segs,
        "kb": [int(x) for x in kb],
        "rows_k": [int(x) for x in rows_k],
        "off_k": [int(x) for x in off_k],
        "nblk": nblk,
        "ngroups": ngroups,
        "ngroups2": ngroups2,
        "pos": pos,
    }
    return idx16, idx16c, dst_t, dst2_t, w_t, w2_t, plan


def _build(n_nodes, hid, plan, n_cores, n_queues=4):
    """Build the SPMD Bass program from the edge plan."""
    shard = n_nodes // n_cores
    nblk = plan["nblk"]
    ngroups = plan["ngroups"]
    ngroups2 = plan["ngroups2"]
    chunks = plan["chunks"]
    chunks2 = plan["chunks2"]
    blk_of_g = plan["blk_of_g"]
    first_g = plan["first_g"]
    last_g = plan["last_g"]
    blk_of_g2 = plan["blk_of_g2"]
    seg_first = plan["seg_first"]
    seg_last = plan["seg_last"]
    seg_idx_g = plan["seg_idx_g"]
    n_segs = plan["n_segs"]
    kb = plan["kb"]
    rows_k = plan["rows_k"]
    off_k = plan["off_k"]
    split = n_nodes // 2
    h2 = 2 * hid

    nc = bacc.Bacc(
        None,
        num_devices=n_cores,
        num_swdge_queues=n_queues,
        dynamic_dma_scratch_size=16 * BLK * MAX_G_CHUNK,
    )

    x1b = nc.dram_tensor("x1b", [n_nodes, h2], BF16, kind="ExternalInput")
    state_s = nc.dram_tensor("state_s", [shard, hid], BF16, kind="ExternalInput")
    featT_s = nc.dram_tensor("featT_s", [hid, shard], BF16, kind="ExternalInput")
    stateT_s = nc.dram_tensor("stateT_s", [hid, shard], BF16, kind="ExternalInput")
    idx16_d = nc.dram_tensor("idx16", [BLK, 8 * ngroups], I16, kind="ExternalInput")
    idx2_d = nc.dram_tensor("idx2", [BLK, 8 * ngroups2], I16, kind="ExternalInput")
    dst_d = nc.dram_tensor("dst_t", [BLK, ngroups], F32, kind="ExternalInput")
    dst2_d = nc.dram_tensor("dst2_t", [BLK, ngroups2], F32, kind="ExternalInput")
    w_d = nc.dram_tensor("w_t", [BLK, ngroups], F32, kind="ExternalInput")
    w2_d = nc.dram_tensor("w2_t", [BLK, ngroups2], F32, kind="ExternalInput")
    wzr = nc.dram_tensor("wzr", [h2, h2], F32, kind="ExternalInput")
    bzr = nc.dram_tensor("bzr", [1, h2], F32, kind="ExternalInput")
    wc = nc.dram_tensor("wc", [h2, hid], F32, kind="ExternalInput")
    bc = nc.dram_tensor("bc", [1, hid], F32, kind="ExternalInput")
    out = nc.dram_tensor("out", [shard, hid], F32, kind="ExternalOutput")

    y2s = [
        nc.dram_tensor(f"y2s{k}", [rows_k[k], hid], BF16, kind="Internal")
        for k in range(KC)
    ]
    y2f_all = nc.dram_tensor(
        "y2f_all", [n_cores * shard, hid], BF16, kind="Internal",
        addr_space="Shared",
    )

    mx1 = max(g1 - g0 for g0, g1, _ in chunks)
    mx2 = max(g1 - g0 for g0, g1, _ in chunks2)
    qn = [0]

    def next_q():
        q = qn[0]
        qn[0] = (qn[0] + 1) % n_queues
        return q

    def rows_of(b):
        return BLK if b < nblk - 1 else shard - (nblk - 1) * BLK

    with tile.TileContext(nc) as tc:
        with (
            tc.tile_pool(name="const", bufs=1) as const_pool,
            tc.tile_pool(name="store", bufs=1) as store_pool,
            tc.tile_pool(name="msg", bufs=4) as msg_pool,
            tc.tile_pool(name="oh", bufs=10) as oh_pool,
            tc.tile_pool(name="blk", bufs=6) as blk_pool,
            tc.tile_pool(name="agg_ps", bufs=SB_BLOCKS + 1, space="PSUM") as agg_psum,
            tc.tile_pool(name="mm_ps", bufs=2, space="PSUM") as mm_psum,
        ):
            nc.gpsimd.load_library(mlp)
            # ---- phase-A-critical tables first (head of the DMA queue) ----
            idx16_sb = store_pool.tile([BLK, 8 * ngroups], I16)
            nc.sync.dma_start(out=idx16_sb[:], in_=idx16_d[:, :])
            dst_sb = store_pool.tile([BLK, ngroups], F32)
            nc.sync.dma_start(out=dst_sb[:], in_=dst_d[:, :])
            w_sb = store_pool.tile([BLK, ngroups], F32)
            nc.sync.dma_start(out=w_sb[:], in_=w_d[:, :])
            # ---- constants ----
            iota_i = const_pool.tile([BLK, BLK], mybir.dt.int32)
            nc.gpsimd.iota(iota_i[:], pattern=[[1, BLK]], base=0, channel_multiplier=0)
            iota_h = const_pool.tile([BLK, BLK], BF16)
            nc.vector.tensor_copy(iota_h[:], iota_i[:])
            iota2_i = const_pool.tile([BLK, 2 * BLK], mybir.dt.int32)
            nc.gpsimd.iota(
                iota2_i[:], pattern=[[1, 2 * BLK]], base=0, channel_multiplier=0
            )
            iota2_h = const_pool.tile([BLK, 2 * BLK], BF16)
            nc.vector.tensor_copy(iota2_h[:], iota2_i[:])
            ones1 = const_pool.tile([1, BLK], F32)
            nc.vector.memset(ones1[:], 1.0)
            wzr_sb = const_pool.tile([h2, h2], F32)
            nc.sync.dma_start(out=wzr_sb[:], in_=wzr[:, :])
            bzr_sb = const_pool.tile([1, h2], F32)
            nc.sync.dma_start(out=bzr_sb[:], in_=bzr[:, :])
            wct_f32 = const_pool.tile([hid, hid], F32)
            nc.sync.dma_start(out=wct_f32[:], in_=wc[0:hid, :])
            wcb_f32 = const_pool.tile([hid, hid], F32)
            nc.sync.dma_start(out=wcb_f32[:], in_=wc[hid:h2, :])
            wctop_sb = const_pool.tile([hid, hid], BF16)
            nc.vector.tensor_copy(wctop_sb[:], wct_f32[:])
            wcbot_sb = const_pool.tile([hid, hid], BF16)
            nc.vector.tensor_copy(wcbot_sb[:], wcb_f32[:])
            bc_sb = const_pool.tile([1, hid], F32)
            nc.sync.dma_start(out=bc_sb[:], in_=bc[:, :])

            # ---- persistent stores (phase-C tables loaded later) ----
            idx2_sb = store_pool.tile([BLK, 8 * ngroups2], I16)
            dst2_sb = store_pool.tile([BLK, ngroups2], F32)
            w2_sb = store_pool.tile([BLK, ngroups2], F32)

            nfull = (nblk - 1) * BLK  # rows in full blocks
            featT_store = store_pool.tile([hid, nblk * BLK], BF16)
            nc.vector.memset(featT_store[:, shard : nblk * BLK], 0.0)
            nc.sync.dma_start(out=featT_store[:, 0:shard], in_=featT_s[:, :])
            stateT_store = store_pool.tile([hid, nblk * BLK], BF16)
            nc.vector.memset(stateT_store[:, shard : nblk * BLK], 0.0)
            nc.sync.dma_start(out=stateT_store[:, 0:shard], in_=stateT_s[:, :])
            st_store = store_pool.tile([BLK, nblk * hid], BF16)
            nc.vector.memset(st_store[:, (nblk - 1) * hid : nblk * hid], 0.0)
            nc.sync.dma_start(
                out=st_store[:, 0 : (nblk - 1) * hid].rearrange(
                    "p (b h) -> p b h", h=hid
                ),
                in_=state_s[0:nfull, :].rearrange("(b p) h -> p b h", p=BLK),
            )
            nc.sync.dma_start(
                out=st_store[: shard - nfull, (nblk - 1) * hid : nblk * hid],
                in_=state_s[nfull:shard, :],
            )
            z_store = store_pool.tile([BLK, nblk * hid], F32)
            acc_store = store_pool.tile([BLK, nblk * hid], F32)

            # ============== Phase A: pass-1 aggregation + y2 ===============
            psum_of = {}
            done_blocks = [0]
            coll_emitted = [0]

            def tail_a(b):
                """Post-aggregation per-block work for pass 1."""
                R = rows_of(b)
                k = int(np.searchsorted(kb[1:], b, side="right"))
                aggT_ps = psum_of.pop(b)
                aggT = blk_pool.tile([h2, BLK], F32, tag="aggT")
                nc.vector.tensor_copy(aggT[:], aggT_ps[:])
                zr_ps = mm_psum.tile([BLK, hid], F32, tag="mm")
                nc.tensor.matmul(
                    zr_ps[:], lhsT=aggT[:], rhs=wzr_sb[:, 0:hid], start=True, stop=False
                )
                nc.tensor.matmul(
                    zr_ps[:], lhsT=ones1[:], rhs=bzr_sb[:, 0:hid], start=False, stop=True
                )
                nc.scalar.activation(
                    z_store[:, b * hid : (b + 1) * hid],
                    zr_ps[:],
                    mybir.ActivationFunctionType.Sigmoid,
                )
                rT_ps = mm_psum.tile([hid, BLK], F32, tag="mm")
                nc.tensor.matmul(
                    rT_ps[:], lhsT=wzr_sb[:, hid:h2], rhs=aggT[:], start=True, stop=False
                )
                nc.tensor.matmul(
                    rT_ps[:], lhsT=bzr_sb[:, hid:h2], rhs=ones1[:], start=False, stop=True
                )
                rT_sb = blk_pool.tile([hid, BLK], BF16, tag="rT")
                nc.scalar.activation(
                    rT_sb[:], rT_ps[:], mybir.ActivationFunctionType.Sigmoid
                )
                rsT = blk_pool.tile([hid, BLK], BF16, tag="rsT")
                nc.vector.tensor_tensor(
                    out=rsT[:],
                    in0=rT_sb[:],
                    in1=stateT_store[:, b * BLK : (b + 1) * BLK],
                    op=mybir.AluOpType.mult,
                )
                y2_ps = mm_psum.tile([BLK, hid], F32, tag="mm")
                nc.tensor.matmul(
                    y2_ps[:],
                    lhsT=featT_store[:, b * BLK : (b + 1) * BLK],
                    rhs=wctop_sb[:],
                    start=True,
                    stop=False,
                )
                nc.tensor.matmul(
                    y2_ps[:], lhsT=rsT[:], rhs=wcbot_sb[:], start=False, stop=True
                )
                y2_sb = blk_pool.tile([BLK, hid], BF16, tag="y2")
                nc.vector.tensor_copy(y2_sb[:], y2_ps[:])
                r0 = b * BLK - kb[k] * BLK  # row offset within source chunk k
                with tc.high_priority():
                    nc.sync.dma_start(out=y2s[k][r0 : r0 + R, :], in_=y2_sb[:R, :])
                done_blocks[0] += 1

            def maybe_emit_colls():
                while coll_emitted[0] < KC and done_blocks[0] >= kb[coll_emitted[0] + 1]:
                    k = coll_emitted[0]
                    with tc.high_priority():
                        nc.gpsimd.collective_compute(
                            "AllGather",
                            mybir.AluOpType.bypass,
                            replica_groups=[list(range(n_cores))],
                            ins=[y2s[k][:, :]],
                            outs=[
                                y2f_all[
                                    8 * off_k[k] : 8 * off_k[k] + n_cores * rows_k[k],
                                    :,
                                ]
                            ],
                        )
                    coll_emitted[0] += 1

            for g0, g1, c in chunks:
                kg = g1 - g0
                nidx = kg * BLK
                tbl = x1b[0:split, :] if c == 0 else x1b[split:n_nodes, :]
                msgs = msg_pool.tile([BLK, max(mx1, mx2) * h2], BF16, tag="m1")
                out_ap = msgs[:, : kg * h2].rearrange("p (t w) -> p t w", w=h2)
                nc.gpsimd.dma_gather(
                    out_ap,
                    tbl,
                    idx16_sb[:, 8 * g0 : 8 * g1],
                    nidx,
                    nidx,
                    h2,
                    queue_num=next_q(),
                    single_packet=False,
                )
                for g in range(g0, g1):
                    b = blk_of_g[g]
                    if b not in psum_of:
                        psum_of[b] = agg_psum.tile(
                            [h2, BLK], F32, tag="agg", name=f"agga{b}"
                        )
                    oh = oh_pool.tile([BLK, BLK], BF16, tag="oh")
                    nc.vector.tensor_scalar(
                        out=oh[:],
                        in0=iota_h[:],
                        scalar1=dst_sb[:, g : g + 1],
                        scalar2=w_sb[:, g : g + 1],
                        op0=mybir.AluOpType.is_equal,
                        op1=mybir.AluOpType.mult,
                    )
                    gl = (g - g0) * h2
                    nc.tensor.matmul(
                        out=psum_of[b][:],
                        lhsT=msgs[:, gl : gl + h2],
                        rhs=oh[:],
                        start=(g == first_g[b]),
                        stop=(g == last_g[b]),
                    )
                    if g == last_g[b]:
                        tail_a(b)
                        maybe_emit_colls()

            # ============== Phase C: pass-2 sweeps over source chunks =======
            def acc_c(b, si, psum_c):
                """Fold segment-si partial aggregate for block b into SBUF/output."""
                R = rows_of(b)
                sl = slice(b * hid, (b + 1) * hid)
                ns_b = int(n_segs[b])
                if si == 0 and ns_b > 1:
                    nc.vector.tensor_copy(acc_store[:, sl], psum_c[:])
                    return
                if si < ns_b - 1:
                    nc.vector.tensor_tensor(
                        out=acc_store[:, sl],
                        in0=psum_c[:],
                        in1=acc_store[:, sl],
                        op=mybir.AluOpType.add,
                    )
                    return
                t0 = blk_pool.tile([BLK, hid], F32, tag="t0")
                if ns_b == 1:
                    nc.vector.tensor_copy(t0[:], psum_c[:])
                else:
                    nc.vector.tensor_tensor(
                        out=t0[:], in0=psum_c[:], in1=acc_store[:, sl],
                        op=mybir.AluOpType.add,
                    )
                c_sb = blk_pool.tile([BLK, hid], F32, tag="c")
                nc.scalar.activation(
                    c_sb[:], t0[:], mybir.ActivationFunctionType.Tanh
                )
                # new_state = c + z*(state - c)
                t1 = blk_pool.tile([BLK, hid], F32, tag="t1")
                nc.gpsimd.tensor_tensor(
                    out=t1[:],
                    in0=st_store[:, sl],
                    in1=c_sb[:],
                    op=mybir.AluOpType.subtract,
                )
                t2 = blk_pool.tile([BLK, hid], F32, tag="t2")
                nc.gpsimd.tensor_tensor(
                    out=t2[:],
                    in0=t1[:],
                    in1=z_store[:, sl],
                    op=mybir.AluOpType.mult,
                )
                ns = blk_pool.tile([BLK, hid], F32, tag="ns")
                nc.gpsimd.tensor_tensor(
                    out=ns[:], in0=t2[:], in1=c_sb[:], op=mybir.AluOpType.add
                )
                nc.sync.dma_start(out=out[b * BLK : b * BLK + R, :], in_=ns[:R, :])

            nc.sync.dma_start(out=idx2_sb[:], in_=idx2_d[:, :])
            nc.sync.dma_start(out=dst2_sb[:], in_=dst2_d[:, :])
            nc.sync.dma_start(out=w2_sb[:], in_=w2_d[:, :])

            psum_c_of = {}
            for g0, g1, cls in chunks2:
                kg = g1 - g0
                nidx = kg * BLK
                tbl = y2f_all[0 : 8 * off_k[cls + 1], :].rearrange(
                    "(n two) h -> n (two h)", two=2
                )
                msgs2 = msg_pool.tile([BLK, max(mx1, mx2) * h2], BF16, tag="m1")
                out_ap = msgs2[:, : kg * h2].rearrange("p (t w) -> p t w", w=h2)
                nc.gpsimd.dma_gather(
                    out_ap,
                    tbl,
                    idx2_sb[:, 8 * g0 : 8 * g1],
                    nidx,
                    nidx,
                    h2,
                    queue_num=next_q(),
                    single_packet=False,
                )
                for g in range(g0, g1):
                    b = blk_of_g2[g]
                    si = int(seg_idx_g[g])
                    final = si == int(n_segs[b]) - 1
                    if b not in psum_c_of:
                        psum_c_of[b] = agg_psum.tile(
                            [BLK, hid], F32, tag="agg", name=f"aggc{b}s{si}"
                        )
                    ohp = oh_pool.tile([BLK, 2 * BLK], BF16, tag="ohp")
                    nc.vector.tensor_scalar(
                        out=ohp[:],
                        in0=iota2_h[:],
                        scalar1=dst2_sb[:, g : g + 1],
                        scalar2=w2_sb[:, g : g + 1],
                        op0=mybir.AluOpType.is_equal,
                        op1=mybir.AluOpType.mult,
                    )
                    gl = (g - g0) * h2
                    nc.tensor.matmul(
                        out=psum_c_of[b][:],
                        lhsT=ohp[:, 0:BLK],
                        rhs=msgs2[:, gl : gl + hid],
                        start=bool(seg_first[g]),
                        stop=False,
                    )
                    nc.tensor.matmul(
                        out=psum_c_of[b][:],
                        lhsT=ohp[:, BLK : 2 * BLK],
                        rhs=msgs2[:, gl + hid : gl + h2],
                        start=False,
                        stop=(bool(seg_last[g]) and not final),
                    )
                    if seg_last[g]:
                        psum_c = psum_c_of.pop(b)
                        if final:
                            nc.tensor.matmul(
                                psum_c[:], lhsT=ones1[:], rhs=bc_sb[:],
                                start=False, stop=True,
                            )
                        acc_c(b, si, psum_c)

    nc.finalize()
    return nc


def run(feat, state, src, dst, edge_weight, Wzr, bzr, Wc, bc, trace=False):
    """Build + run on 8 cores; returns (new_state, BassKernelResults)."""
    n_nodes, hid = feat.shape
    n_cores = N_CORES
    shard = n_nodes // n_cores

    idx16, idx16c, dst_t, dst2_t, w_t, w2_t, plan = _prep_edges(
        dst, src, edge_weight, n_nodes, n_cores
    )
    import ml_dtypes

    pos = plan["pos"]
    # global permutation: node (p, l) lives at row p*shard + pos[p, l]
    inv = np.empty((n_cores, shard), np.int64)
    for p in range(n_cores):
        inv[p, pos[p]] = np.arange(shard)
    x1 = np.concatenate([feat, state], axis=1)
    x1p = np.empty_like(x1)
    for p in range(n_cores):
        x1p[p * shard : (p + 1) * shard] = x1[p * shard : (p + 1) * shard][inv[p]]
    x1b = np.ascontiguousarray(x1p.astype(ml_dtypes.bfloat16))

    nc = _build(n_nodes, hid, plan, n_cores)

    in_maps = []
    for p in range(n_cores):
        feat_p = feat[p * shard : (p + 1) * shard][inv[p]]
        state_p = state[p * shard : (p + 1) * shard][inv[p]]
        in_maps.append(
            {
                "x1b": x1b,
                "state_s": np.ascontiguousarray(state_p.astype(ml_dtypes.bfloat16)),
                "featT_s": np.ascontiguousarray(feat_p.T.astype(ml_dtypes.bfloat16)),
                "stateT_s": np.ascontiguousarray(state_p.T.astype(ml_dtypes.bfloat16)),
                "idx16": np.ascontiguousarray(idx16[p]),
                "idx2": np.ascontiguousarray(idx16c[p]),
                "dst_t": np.ascontiguousarray(dst_t[p]),
                "dst2_t": np.ascontiguousarray(dst2_t[p]),
                "w_t": np.ascontiguousarray(w_t[p]),
                "w2_t": np.ascontiguousarray(w2_t[p]),
                "wzr": np.ascontiguousarray(Wzr, dtype=np.float32),
                "bzr": np.ascontiguousarray(bzr.reshape(1, -1), dtype=np.float32),
                "wc": np.ascontiguousarray(Wc, dtype=np.float32),
                "bc": np.ascontiguousarray(bc.reshape(1, -1), dtype=np.float32),
            }
        )

    res = run_bass_kernel_spmd(
        nc, in_maps, core_ids=list(range(n_cores)), trace=trace
    )
    shards = [res.results[p]["out"][pos[p]] for p in range(n_cores)]
    return np.concatenate(shards, axis=0), res


def kernel(feat, state, src, dst, edge_weight, Wzr, bzr, Wc, bc):
    out, _ = run(feat, state, src, dst, edge_weight, Wzr, bzr, Wc, bc, trace=False)
    return out


# revision 57
# speedup vs baseline: 1.0187x; 1.0187x over previous
"""DCGRU cell on 8 Trainium2 NeuronCores.

Strategy (dst-sharded graph partitioning, overlapped AllGather):
  - Nodes are sharded into 8 contiguous ranges (one per core). Edges are
    assigned to the core owning dst; within a core, dst nodes are processed
    in blocks of 128 grouped into super-blocks of SB_BLOCKS for gathering.
  - Pass 1 gathers x1 = [feat, state] rows (bf16, 256B) per edge and
    segment-sums them via one-hot matmuls into PSUM (transposed layout).
    zr/r^T/y2 are produced without PE transposes: the host supplies feat^T
    and state^T so y2 = feat @ Wc_top + (r*state) @ Wc_bot uses featT/rsT
    directly as lhsT.
  - y2 (bf16) is AllGathered in KC block-range chunks, each issued as soon
    as its blocks finish in pass 1, so the collective overlaps phase-A
    compute (collective cost model: 15us fixed + bytes/40GBps).
  - Pass 2 gathers y2 rows (128B) directly, sweeping source-chunks k-outer
    so sweep k only waits on collective chunk k; per (block, sweep) partial
    PSUM aggregates accumulate into an SBUF accumulator.
"""

import numpy as np

import concourse.bass as bass
import concourse.bacc as bacc
import concourse.mybir as mybir
import concourse.tile as tile
from concourse.bass_utils import run_bass_kernel_spmd
from concourse.library_config import mlp
from concourse.masks import make_identity

N_NODES = 50000
N_EDGES = 640000
HID = 64
N_CORES = 8
BLK = 128           # dst nodes per block (= PSUM partition dim)
SB_BLOCKS = 4       # dst blocks per super-block (PSUM live tiles = SB_BLOCKS)
MAX_G_CHUNK = 32    # cap on groups per dma_gather instruction
KC = 4              # source chunks for the AllGather / pass-2 sweeps

F32 = mybir.dt.float32
BF16 = mybir.dt.bfloat16
I16 = mybir.dt.int16


def _kchunk_blocks(nblk):
    """Block ranges per source chunk: decreasing sizes so each sweep's
    collective chunk lands just before the sweep needs it."""
    if nblk == 49 and KC == 4:
        sizes = [4, 22, 15, 8]
    else:
        base = nblk // KC
        rem = nblk % KC
        sizes = [base + (1 if i < rem else 0) for i in range(KC)]
    bounds = np.concatenate([[0], np.cumsum(sizes)])
    return bounds  # len KC+1, bounds[-1] == nblk


def _build_tables(e_owner, e_ukey, e_idxval, e_dloc, e_w, unit_off, n_units):
    """Slot edges into (group, lane) and build idx16/dst/w tables.

    e_* are per-edge arrays; e_ukey is the unit id (dense, 0..n_units-1);
    e_idxval is the int16 gather index value. Returns (idx16, dst_t, w_t).
    """
    e = len(e_owner)
    ngroups = int(unit_off[-1])
    ck = e_owner * n_units + e_ukey
    order = np.argsort(ck, kind="stable")
    ck_s = ck[order]
    owner_s = e_owner[order]
    idxval_s = e_idxval[order]
    ukey_s = e_ukey[order]
    dloc_s = e_dloc[order]
    w_s = e_w[order]
    bucket_start = np.searchsorted(ck_s, np.arange(N_CORES * n_units))
    rank = np.arange(e) - bucket_start[ck_s]
    g_global = unit_off[ukey_s] + rank // BLK
    lane = rank % BLK

    idx16 = np.zeros((N_CORES, 16, 8 * ngroups), np.int16)
    dst_t = np.zeros((N_CORES, BLK, ngroups), np.float32)
    w_t = np.zeros((N_CORES, BLK, ngroups), np.float32)
    idx16[owner_s, lane % 16, 8 * g_global + lane // 16] = idxval_s.astype(np.int16)
    dst_t[owner_s, lane, g_global] = dloc_s
    w_t[owner_s, lane, g_global] = w_s
    idx16 = np.tile(idx16, (1, 8, 1))
    return idx16, dst_t, w_t


def _prep_edges(dst, src, edge_weight, n_nodes, n_cores):
    """Partition edges by dst core/block; build pass-1 and pass-2 tables."""
    shard = n_nodes // n_cores
    nblk = (shard + BLK - 1) // BLK
    split = n_nodes // 2
    e = len(dst)

    dsts = dst.astype(np.int64)
    srcs = src.astype(np.int64)
    owner = dsts // shard
    local = dsts - owner * shard

    # Balance in-degree across blocks: per core, deal nodes (sorted by
    # in-degree, desc) round-robin over blocks. pos[core, orig_local] is the
    # node's new row; node tables / shard rows / outputs use this order.
    deg = np.zeros(n_nodes, np.int64)
    np.add.at(deg, dsts, 1)
    pos = np.empty((n_cores, shard), np.int64)
    blk_fill = np.empty(nblk, np.int64)
    cap = np.full(nblk, BLK, np.int64)
    cap[nblk - 1] = shard - (nblk - 1) * BLK
    for p in range(n_cores):
        nodes = np.argsort(-deg[p * shard : (p + 1) * shard], kind="stable")
        blk_fill[:] = 0
        bi = 0
        for n in nodes:
            while blk_fill[bi % nblk] >= cap[bi % nblk]:
                bi += 1
            b = bi % nblk
            pos[p, n] = b * BLK + blk_fill[b]
            blk_fill[b] += 1
            bi += 1
    newloc = pos[owner, local]
    b_of = newloc // BLK
    dloc = (newloc % BLK).astype(np.float32)
    w_f = edge_weight.astype(np.float32)

    # permuted global src position
    src_pos = pos[srcs // shard, srcs % shard] + (srcs // shard) * shard

    # ---------------- pass 1: units = (super-block, class, block) ----------
    cls = (src_pos >= split).astype(np.int64)
    src_local1 = src_pos - cls * split

    cnt = np.zeros((n_cores, nblk, 2), np.int64)
    np.add.at(cnt, (owner, b_of, cls), 1)
    gpbc = -(-cnt.max(axis=0) // BLK)  # [nblk, 2]
    empty = gpbc.sum(axis=1) == 0
    gpbc[empty, 0] = 1

    unit_order = []
    for sb0 in range(0, nblk, SB_BLOCKS):
        sbb = range(sb0, min(sb0 + SB_BLOCKS, nblk))
        for c in range(2):
            for b in sbb:
                unit_order.append((b, c))
    unit_sizes = np.array([gpbc[b, c] for b, c in unit_order], np.int64)
    unit_off = np.concatenate([[0], np.cumsum(unit_sizes)])
    ngroups = int(unit_off[-1])
    unit_idx = {bc: i for i, bc in enumerate(unit_order)}

    blk_of_g = np.zeros(ngroups, np.int64)
    for i, (b, c) in enumerate(unit_order):
        blk_of_g[unit_off[i] : unit_off[i + 1]] = b
    first_g = np.full(nblk, -1, np.int64)
    last_g = np.full(nblk, -1, np.int64)
    for g in range(ngroups):
        b = blk_of_g[g]
        if first_g[b] < 0:
            first_g[b] = g
        last_g[b] = g

    # gather chunks: contiguous same-class unit runs within a super-block
    chunks = []
    i = 0
    while i < len(unit_order):
        c = unit_order[i][1]
        sb = unit_order[i][0] // SB_BLOCKS
        j = i
        while (
            j < len(unit_order)
            and unit_order[j][1] == c
            and unit_order[j][0] // SB_BLOCKS == sb
        ):
            j += 1
        g0, g1 = int(unit_off[i]), int(unit_off[j])
        for s in range(g0, g1, MAX_G_CHUNK):
            if s < g1:
                chunks.append((s, min(s + MAX_G_CHUNK, g1), c))
        i = j

    ukey1 = np.array([unit_idx[(b, c)] for b, c in zip(b_of, cls)], np.int64)
    idx16, dst_t, w_t = _build_tables(
        owner, ukey1, src_local1, dloc, w_f, unit_off, len(unit_order)
    )

    # ---------------- pass 2: units = (kchunk, block) ----------------------
    kb = _kchunk_blocks(nblk)  # block bounds, len KC+1
    krow = kb * BLK  # row bounds within shard (last may exceed shard)
    krow[-1] = shard
    rows_k = np.diff(krow)  # rows per chunk per core

    assert all(r % 2 == 0 for r in rows_k), "pair trick needs even chunk rows"
    off_k = np.concatenate([[0], np.cumsum(rows_k)])  # per-core chunk offsets
    sp_core = src_pos // shard
    sp_local = src_pos - sp_core * shard
    sp_blk = sp_local // BLK
    kc_of = np.searchsorted(kb[1:], sp_blk, side="right")
    # row in the chunk-major y2f_all tensor
    crow = 8 * off_k[kc_of] + sp_core * rows_k[kc_of] + (sp_local - krow[kc_of])
    idxval2 = crow // 2
    par2 = (crow % 2).astype(np.float32)

    # groups per dst block (edges k-sorted within each (core, block))
    cnt2 = np.zeros((n_cores, nblk), np.int64)
    np.add.at(cnt2, (owner, b_of), 1)
    gpb = np.maximum(-(-cnt2.max(axis=0) // BLK), 1)  # [nblk]

    # per-core rank within (block), with edges sorted by kc inside the block
    ck2 = (owner * nblk + b_of) * (KC + 1) + kc_of
    order2 = np.argsort(ck2, kind="stable")
    cb_key = (owner * nblk + b_of)[order2]
    kc_s = kc_of[order2]
    bucket2 = np.searchsorted(cb_key, np.arange(n_cores * nblk))
    rank2 = np.arange(e) - bucket2[cb_key]
    j_of = rank2 // BLK  # within-block group index

    # class of within-block group j = max over cores of kc at that rank range
    cls_jb = np.zeros((nblk, int(gpb.max())), np.int64)
    np.maximum.at(cls_jb, ((b_of[order2]), j_of), kc_s)

    # global group order: (class, block, j)
    tuples = []
    for b in range(nblk):
        for j in range(int(gpb[b])):
            tuples.append((int(cls_jb[b, j]), b, j))
    tuples.sort()
    ngroups2 = len(tuples)
    g_of = np.zeros((nblk, int(gpb.max())), np.int64)
    blk_of_g2 = np.zeros(ngroups2, np.int64)
    cls_of_g2 = np.zeros(ngroups2, np.int64)
    for g, (c, b, j) in enumerate(tuples):
        g_of[b, j] = g
        blk_of_g2[g] = b
        cls_of_g2[g] = c

    # per-block segments (contiguous same-class group runs in global order)
    seg_first = np.zeros(ngroups2, bool)
    seg_last = np.zeros(ngroups2, bool)
    seg_idx_g = np.zeros(ngroups2, np.int64)
    n_segs = np.zeros(nblk, np.int64)
    for b in range(nblk):
        gs = sorted(g_of[b, : int(gpb[b])])
        si = 0
        for i, g in enumerate(gs):
            if i == 0 or cls_of_g2[g] != cls_of_g2[gs[i - 1]]:
                seg_first[g] = True
                if i > 0:
                    seg_last[gs[i - 1]] = True
                    si += 1
            seg_idx_g[g] = si
        seg_last[gs[-1]] = True
        n_segs[b] = si + 1

    # gather chunks: same-class runs capped at MAX_G_CHUNK
    chunks2 = []
    i = 0
    while i < ngroups2:
        c = int(cls_of_g2[i])
        j = i
        while j < ngroups2 and cls_of_g2[j] == c:
            j += 1
        for s in range(i, j, MAX_G_CHUNK):
            chunks2.append((s, min(s + MAX_G_CHUNK, j), c))
        i = j

    # build tables with per-edge global group/lane
    owner_s2 = owner[order2]
    g_global2 = g_of[b_of[order2], j_of]
    lane2 = rank2 % BLK
    idxval2_s = idxval2[order2]
    dstp2_s = (dloc + BLK * par2)[order2]
    w2_s = w_f[order2]
    idx16c = np.zeros((N_CORES, 16, 8 * ngroups2), np.int16)
    dst2_t = np.zeros((N_CORES, BLK, ngroups2), np.float32)
    w2_t = np.zeros((N_CORES, BLK, ngroups2), np.float32)
    idx16c[owner_s2, lane2 % 16, 8 * g_global2 + lane2 // 16] = idxval2_s.astype(
        np.int16
    )
    dst2_t[owner_s2, lane2, g_global2] = dstp2_s
    w2_t[owner_s2, lane2, g_global2] = w2_s
    idx16c = np.tile(idx16c, (1, 8, 1))

    plan = {
        "chunks": chunks,
        "chunks2": chunks2,
        "blk_of_g": [int(x) for x in blk_of_g],
        "first_g": [int(x) for x in first_g],
        "last_g": [int(x) for x in last_g],
        "blk_of_g2": [int(x) for x in blk_of_g2],
        "seg_first": seg_first,
        "seg_last": seg_last,
        "seg_idx_g": seg_idx_g,
        "n_segs": n_segs,
        "kb": [int(x) for x in kb],
        "rows_k": [int(x) for x in rows_k],
        "off_k": [int(x) for x in off_k],
        "nblk": nblk,
        "ngroups": ngroups,
        "ngroups2": ngroups2,
        "pos": pos,
    }
    return idx16, idx16c, dst_t, dst2_t, w_t, w2_t, plan


def _build(n_nodes, hid, plan, n_cores, n_queues=4):
    """Build the SPMD Bass program from the edge plan."""
    shard = n_nodes // n_cores
    nblk = plan["nblk"]
    ngroups = plan["ngroups"]
    ngroups2 = plan["ngroups2"]
    chunks = plan["chunks"]
    chunks2 = plan["chunks2"]
    blk_of_g = plan["blk_of_g"]
    first_g = plan["first_g"]
    last_g = plan["last_g"]
    blk_of_g2 = plan["blk_of_g2"]
    seg_first = plan["seg_first"]
    seg_last = plan["seg_last"]
    seg_idx_g = plan["seg_idx_g"]
    n_segs = plan["n_segs"]
    kb = plan["kb"]
    rows_k = plan["rows_k"]
    off_k = plan["off_k"]
    split = n_nodes // 2
    h2 = 2 * hid

    nc = bacc.Bacc(
        None,
        num_devices=n_cores,
        num_swdge_queues=n_queues,
        dynamic_dma_scratch_size=16 * BLK * MAX_G_CHUNK,
    )

    x1b = nc.dram_tensor("x1b", [n_nodes, h2], BF16, kind="ExternalInput")
    state_s = nc.dram_tensor("state_s", [shard, hid], BF16, kind="ExternalInput")
    featT_s = nc.dram_tensor("featT_s", [hid, shard], BF16, kind="ExternalInput")
    stateT_s = nc.dram_tensor("stateT_s", [hid, shard], BF16, kind="ExternalInput")
    idx16_d = nc.dram_tensor("idx16", [BLK, 8 * ngroups], I16, kind="ExternalInput")
    idx2_d = nc.dram_tensor("idx2", [BLK, 8 * ngroups2], I16, kind="ExternalInput")
    dst_d = nc.dram_tensor("dst_t", [BLK, ngroups], F32, kind="ExternalInput")
    dst2_d = nc.dram_tensor("dst2_t", [BLK, ngroups2], F32, kind="ExternalInput")
    w_d = nc.dram_tensor("w_t", [BLK, ngroups], F32, kind="ExternalInput")
    w2_d = nc.dram_tensor("w2_t", [BLK, ngroups2], F32, kind="ExternalInput")
    wzr = nc.dram_tensor("wzr", [h2, h2], F32, kind="ExternalInput")
    bzr = nc.dram_tensor("bzr", [1, h2], F32, kind="ExternalInput")
    wc = nc.dram_tensor("wc", [h2, hid], F32, kind="ExternalInput")
    bc = nc.dram_tensor("bc", [1, hid], F32, kind="ExternalInput")
    out = nc.dram_tensor("out", [shard, hid], F32, kind="ExternalOutput")

    y2s = [
        nc.dram_tensor(f"y2s{k}", [rows_k[k], hid], BF16, kind="Internal")
        for k in range(KC)
    ]
    y2f_all = nc.dram_tensor(
        "y2f_all", [n_cores * shard, hid], BF16, kind="Internal",
        addr_space="Shared",
    )

    mx1 = max(g1 - g0 for g0, g1, _ in chunks)
    mx2 = max(g1 - g0 for g0, g1, _ in chunks2)
    qn = [0]

    def next_q():
        q = qn[0]
        qn[0] = (qn[0] + 1) % n_queues
        return q

    def rows_of(b):
        return BLK if b < nblk - 1 else shard - (nblk - 1) * BLK

    with tile.TileContext(nc) as tc:
        with (
            tc.tile_pool(name="const", bufs=1) as const_pool,
            tc.tile_pool(name="store", bufs=1) as store_pool,
            tc.tile_pool(name="msg", bufs=4) as msg_pool,
            tc.tile_pool(name="oh", bufs=10) as oh_pool,
            tc.tile_pool(name="blk", bufs=6) as blk_pool,
            tc.tile_pool(name="agg_ps", bufs=SB_BLOCKS + 1, space="PSUM") as agg_psum,
            tc.tile_pool(name="mm_ps", bufs=2, space="PSUM") as mm_psum,
        ):
            nc.gpsimd.load_library(mlp)
            # ---- phase-A-critical tables first (head of the DMA queue) ----
            idx16_sb = store_pool.tile([BLK, 8 * ngroups], I16)
            nc.sync.dma_start(out=idx16_sb[:], in_=idx16_d[:, :])
            dst_sb = store_pool.tile([BLK, ngroups], F32)
            nc.sync.dma_start(out=dst_sb[:], in_=dst_d[:, :])
            w_sb = store_pool.tile([BLK, ngroups], F32)
            nc.sync.dma_start(out=w_sb[:], in_=w_d[:, :])
            # ---- constants ----
            iota_i = const_pool.tile([BLK, BLK], mybir.dt.int32)
            nc.gpsimd.iota(iota_i[:], pattern=[[1, BLK]], base=0, channel_multiplier=0)
            iota_h = const_pool.tile([BLK, BLK], BF16)
            nc.vector.tensor_copy(iota_h[:], iota_i[:])
            iota2_i = const_pool.tile([BLK, 2 * BLK], mybir.dt.int32)
            nc.gpsimd.iota(
                iota2_i[:], pattern=[[1, 2 * BLK]], base=0, channel_multiplier=0
            )
            iota2_h = const_pool.tile([BLK, 2 * BLK], BF16)
            nc.vector.tensor_copy(iota2_h[:], iota2_i[:])
            ones1 = const_pool.tile([1, BLK], F32)
            nc.vector.memset(ones1[:], 1.0)
            wzr_sb = const_pool.tile([h2, h2], F32)
            nc.sync.dma_start(out=wzr_sb[:], in_=wzr[:, :])
            bzr_sb = const_pool.tile([1, h2], F32)
            nc.sync.dma_start(out=bzr_sb[:], in_=bzr[:, :])
            wct_f32 = const_pool.tile([hid, hid], F32)
            nc.sync.dma_start(out=wct_f32[:], in_=wc[0:hid, :])
            wcb_f32 = const_pool.tile([hid, hid], F32)
            nc.sync.dma_start(out=wcb_f32[:], in_=wc[hid:h2, :])
            wctop_sb = const_pool.tile([hid, hid], BF16)
            nc.vector.tensor_copy(wctop_sb[:], wct_f32[:])
            wcbot_sb = const_pool.tile([hid, hid], BF16)
            nc.vector.tensor_copy(wcbot_sb[:], wcb_f32[:])
            bc_sb = const_pool.tile([1, hid], F32)
            nc.sync.dma_start(out=bc_sb[:], in_=bc[:, :])

            # ---- persistent stores (phase-C tables loaded later) ----
            idx2_sb = store_pool.tile([BLK, 8 * ngroups2], I16)
            dst2_sb = store_pool.tile([BLK, ngroups2], F32)
            w2_sb = store_pool.tile([BLK, ngroups2], F32)

            nfull = (nblk - 1) * BLK  # rows in full blocks
            featT_store = store_pool.tile([hid, nblk * BLK], BF16)
            nc.vector.memset(featT_store[:, shard : nblk * BLK], 0.0)
            nc.sync.dma_start(out=featT_store[:, 0:shard], in_=featT_s[:, :])
            stateT_store = store_pool.tile([hid, nblk * BLK], BF16)
            nc.vector.memset(stateT_store[:, shard : nblk * BLK], 0.0)
            nc.sync.dma_start(out=stateT_store[:, 0:shard], in_=stateT_s[:, :])
            st_store = store_pool.tile([BLK, nblk * hid], BF16)
            nc.vector.memset(st_store[:, (nblk - 1) * hid : nblk * hid], 0.0)
            nc.sync.dma_start(
                out=st_store[:, 0 : (nblk - 1) * hid].rearrange(
                    "p (b h) -> p b h", h=hid
                ),
                in_=state_s[0:nfull, :].rearrange("(b p) h -> p b h", p=BLK),
            )
            nc.sync.dma_start(
                out=st_store[: shard - nfull, (nblk - 1) * hid : nblk * hid],
                in_=state_s[nfull:shard, :],
            )
            z_store = store_pool.tile([BLK, nblk * hid], F32)
            acc_store = store_pool.tile([BLK, nblk * hid], F32)
            y2_store = store_pool.tile([BLK, nblk * hid], BF16)

            # ============== Phase A: pass-1 aggregation + y2 ===============
            psum_of = {}
            done_blocks = [0]
            coll_emitted = [0]

            def tail_a(b):
                """Post-aggregation per-block work for pass 1."""
                R = rows_of(b)
                k = int(np.searchsorted(kb[1:], b, side="right"))
                aggT_ps = psum_of.pop(b)
                aggT = blk_pool.tile([h2, BLK], F32, tag="aggT")
                nc.vector.tensor_copy(aggT[:], aggT_ps[:])
                zr_ps = mm_psum.tile([BLK, hid], F32, tag="mm")
                nc.tensor.matmul(
                    zr_ps[:], lhsT=aggT[:], rhs=wzr_sb[:, 0:hid], start=True, stop=False
                )
                nc.tensor.matmul(
                    zr_ps[:], lhsT=ones1[:], rhs=bzr_sb[:, 0:hid], start=False, stop=True
                )
                nc.scalar.activation(
                    z_store[:, b * hid : (b + 1) * hid],
                    zr_ps[:],
                    mybir.ActivationFunctionType.Sigmoid,
                )
                rT_ps = mm_psum.tile([hid, BLK], F32, tag="mm")
                nc.tensor.matmul(
                    rT_ps[:], lhsT=wzr_sb[:, hid:h2], rhs=aggT[:], start=True, stop=False
                )
                nc.tensor.matmul(
                    rT_ps[:], lhsT=bzr_sb[:, hid:h2], rhs=ones1[:], start=False, stop=True
                )
                rT_sb = blk_pool.tile([hid, BLK], BF16, tag="rT")
                nc.scalar.activation(
                    rT_sb[:], rT_ps[:], mybir.ActivationFunctionType.Sigmoid
                )
                rsT = blk_pool.tile([hid, BLK], BF16, tag="rsT")
                nc.vector.tensor_tensor(
                    out=rsT[:],
                    in0=rT_sb[:],
                    in1=stateT_store[:, b * BLK : (b + 1) * BLK],
                    op=mybir.AluOpType.mult,
                )
                y2_ps = mm_psum.tile([BLK, hid], F32, tag="mm")
                nc.tensor.matmul(
                    y2_ps[:],
                    lhsT=featT_store[:, b * BLK : (b + 1) * BLK],
                    rhs=wctop_sb[:],
                    start=True,
                    stop=False,
                )
                nc.tensor.matmul(
                    y2_ps[:], lhsT=rsT[:], rhs=wcbot_sb[:], start=False, stop=True
                )
                nc.vector.tensor_copy(
                    y2_store[:, b * hid : (b + 1) * hid], y2_ps[:]
                )
                done_blocks[0] += 1

            def maybe_emit_colls():
                while coll_emitted[0] < KC and done_blocks[0] >= kb[coll_emitted[0] + 1]:
                    k = coll_emitted[0]
                    b0, b1 = kb[k], kb[k + 1]
                    nbf = b1 - b0 if b1 < nblk else b1 - b0 - 1
                    with tc.high_priority():
                        if nbf > 0:
                            nc.sync.dma_start(
                                out=y2s[k][0 : nbf * BLK, :].rearrange(
                                    "(b p) h -> p b h", p=BLK
                                ),
                                in_=y2_store[
                                    :, b0 * hid : (b0 + nbf) * hid
                                ].rearrange("p (b h) -> p b h", h=hid),
                            )
                        if b1 == nblk:
                            R = shard - (nblk - 1) * BLK
                            nc.sync.dma_start(
                                out=y2s[k][nbf * BLK : nbf * BLK + R, :],
                                in_=y2_store[
                                    :R, (nblk - 1) * hid : nblk * hid
                                ],
                            )
                    with tc.high_priority():
                        nc.gpsimd.collective_compute(
                            "AllGather",
                            mybir.AluOpType.bypass,
                            replica_groups=[list(range(n_cores))],
                            ins=[y2s[k][:, :]],
                            outs=[
                                y2f_all[
                                    8 * off_k[k] : 8 * off_k[k] + n_cores * rows_k[k],
                                    :,
                                ]
                            ],
                        )
                    coll_emitted[0] += 1

            for g0, g1, c in chunks:
                kg = g1 - g0
                nidx = kg * BLK
                tbl = x1b[0:split, :] if c == 0 else x1b[split:n_nodes, :]
                msgs = msg_pool.tile([BLK, max(mx1, mx2) * h2], BF16, tag="m1")
                out_ap = msgs[:, : kg * h2].rearrange("p (t w) -> p t w", w=h2)
                nc.gpsimd.dma_gather(
                    out_ap,
                    tbl,
                    idx16_sb[:, 8 * g0 : 8 * g1],
                    nidx,
                    nidx,
                    h2,
                    queue_num=next_q(),
                    single_packet=False,
                )
                for g in range(g0, g1):
                    b = blk_of_g[g]
                    if b not in psum_of:
                        psum_of[b] = agg_psum.tile(
                            [h2, BLK], F32, tag="agg", name=f"agga{b}"
                        )
                    oh = oh_pool.tile([BLK, BLK], BF16, tag="oh")
                    nc.vector.tensor_scalar(
                        out=oh[:],
                        in0=iota_h[:],
                        scalar1=dst_sb[:, g : g + 1],
                        scalar2=w_sb[:, g : g + 1],
                        op0=mybir.AluOpType.is_equal,
                        op1=mybir.AluOpType.mult,
                    )
                    gl = (g - g0) * h2
                    nc.tensor.matmul(
                        out=psum_of[b][:],
                        lhsT=msgs[:, gl : gl + h2],
                        rhs=oh[:],
                        start=(g == first_g[b]),
                        stop=(g == last_g[b]),
                    )
                    if g == last_g[b]:
                        tail_a(b)
                        maybe_emit_colls()

            # ============== Phase C: pass-2 sweeps over source chunks =======
            def acc_c(b, si, psum_c):
                """Fold segment-si partial aggregate for block b into SBUF/output."""
                R = rows_of(b)
                sl = slice(b * hid, (b + 1) * hid)
                ns_b = int(n_segs[b])
                if si == 0 and ns_b > 1:
                    nc.vector.tensor_copy(acc_store[:, sl], psum_c[:])
                    return
                if si < ns_b - 1:
                    nc.vector.tensor_tensor(
                        out=acc_store[:, sl],
                        in0=psum_c[:],
                        in1=acc_store[:, sl],
                        op=mybir.AluOpType.add,
                    )
                    return
                t0 = blk_pool.tile([BLK, hid], F32, tag="t0")
                if ns_b == 1:
                    nc.vector.tensor_copy(t0[:], psum_c[:])
                else:
                    nc.vector.tensor_tensor(
                        out=t0[:], in0=psum_c[:], in1=acc_store[:, sl],
                        op=mybir.AluOpType.add,
                    )
                c_sb = blk_pool.tile([BLK, hid], F32, tag="c")
                nc.scalar.activation(
                    c_sb[:], t0[:], mybir.ActivationFunctionType.Tanh
                )
                # new_state = c + z*(state - c)
                t1 = blk_pool.tile([BLK, hid], F32, tag="t1")
                nc.gpsimd.tensor_tensor(
                    out=t1[:],
                    in0=st_store[:, sl],
                    in1=c_sb[:],
                    op=mybir.AluOpType.subtract,
                )
                t2 = blk_pool.tile([BLK, hid], F32, tag="t2")
                nc.gpsimd.tensor_tensor(
                    out=t2[:],
                    in0=t1[:],
                    in1=z_store[:, sl],
                    op=mybir.AluOpType.mult,
                )
                ns = blk_pool.tile([BLK, hid], F32, tag="ns")
                nc.gpsimd.tensor_tensor(
                    out=ns[:], in0=t2[:], in1=c_sb[:], op=mybir.AluOpType.add
                )
                nc.sync.dma_start(out=out[b * BLK : b * BLK + R, :], in_=ns[:R, :])

            nc.sync.dma_start(out=idx2_sb[:], in_=idx2_d[:, :])
            nc.sync.dma_start(out=dst2_sb[:], in_=dst2_d[:, :])
            nc.sync.dma_start(out=w2_sb[:], in_=w2_d[:, :])

            psum_c_of = {}
            for g0, g1, cls in chunks2:
                kg = g1 - g0
                nidx = kg * BLK
                tbl = y2f_all[0 : 8 * off_k[cls + 1], :].rearrange(
                    "(n two) h -> n (two h)", two=2
                )
                msgs2 = msg_pool.tile([BLK, max(mx1, mx2) * h2], BF16, tag="m1")
                out_ap = msgs2[:, : kg * h2].rearrange("p (t w) -> p t w", w=h2)
                nc.gpsimd.dma_gather(
                    out_ap,
                    tbl,
                    idx2_sb[:, 8 * g0 : 8 * g1],
                    nidx,
                    nidx,
                    h2,
                    queue_num=next_q(),
                    single_packet=False,
                )
                for g in range(g0, g1):
                    b = blk_of_g2[g]
                    si = int(seg_idx_g[g])
                    final = si == int(n_segs[b]) - 1
                    if b not in psum_c_of:
                        psum_c_of[b] = agg_psum.tile(
                            [BLK, hid], F32, tag="agg", name=f"aggc{b}s{si}"
                        )
                    ohp = oh_pool.tile([BLK, 2 * BLK], BF16, tag="ohp")
                    nc.vector.tensor_scalar(
                        out=ohp[:],
                        in0=iota2_h[:],
                        scalar1=dst2_sb[:, g : g + 1],
                        scalar2=w2_sb[:, g : g + 1],
                        op0=mybir.AluOpType.is_equal,
                        op1=mybir.AluOpType.mult,
                    )
                    gl = (g - g0) * h2
                    nc.tensor.matmul(
                        out=psum_c_of[b][:],
                        lhsT=ohp[:, 0:BLK],
                        rhs=msgs2[:, gl : gl + hid],
                        start=bool(seg_first[g]),
                        stop=False,
                    )
                    nc.tensor.matmul(
                        out=psum_c_of[b][:],
                        lhsT=ohp[:, BLK : 2 * BLK],
                        rhs=msgs2[:, gl + hid : gl + h2],
                        start=False,
                        stop=(bool(seg_last[g]) and not final),
                    )
                    if seg_last[g]:
                        psum_c = psum_c_of.pop(b)
                        if final:
                            nc.tensor.matmul(
                                psum_c[:], lhsT=ones1[:], rhs=bc_sb[:],
                                start=False, stop=True,
                            )
                        acc_c(b, si, psum_c)

    nc.finalize()
    return nc


def run(feat, state, src, dst, edge_weight, Wzr, bzr, Wc, bc, trace=False):
    """Build + run on 8 cores; returns (new_state, BassKernelResults)."""
    n_nodes, hid = feat.shape
    n_cores = N_CORES
    shard = n_nodes // n_cores

    idx16, idx16c, dst_t, dst2_t, w_t, w2_t, plan = _prep_edges(
        dst, src, edge_weight, n_nodes, n_cores
    )
    import ml_dtypes

    pos = plan["pos"]
    # global permutation: node (p, l) lives at row p*shard + pos[p, l]
    inv = np.empty((n_cores, shard), np.int64)
    for p in range(n_cores):
        inv[p, pos[p]] = np.arange(shard)
    x1 = np.concatenate([feat, state], axis=1)
    x1p = np.empty_like(x1)
    for p in range(n_cores):
        x1p[p * shard : (p + 1) * shard] = x1[p * shard : (p + 1) * shard][inv[p]]
    x1b = np.ascontiguousarray(x1p.astype(ml_dtypes.bfloat16))

    nc = _build(n_nodes, hid, plan, n_cores)

    in_maps = []
    for p in range(n_cores):
        feat_p = feat[p * shard : (p + 1) * shard][inv[p]]
        state_p = state[p * shard : (p + 1) * shard][inv[p]]
        in_maps.append(
            {
                "x1b": x1b,
                "state_s": np.ascontiguousarray(state_p.astype(ml_dtypes.bfloat16)),
                "featT_s": np.ascontiguousarray(feat_p.T.astype(ml_dtypes.bfloat16)),
                "stateT_s": np.ascontiguousarray(state_p.T.astype(ml_dtypes.bfloat16)),
                "idx16": np.ascontiguousarray(idx16[p]),
                "idx2": np.ascontiguousarray(idx16c[p]),
                "dst_t": np.ascontiguousarray(dst_t[p]),
                "dst2_t": np.ascontiguousarray(dst2_t[p]),
                "w_t": np.ascontiguousarray(w_t[p]),
                "w2_t": np.ascontiguousarray(w2_t[p]),
                "wzr": np.ascontiguousarray(Wzr, dtype=np.float32),
                "bzr": np.ascontiguousarray(bzr.reshape(1, -1), dtype=np.float32),
                "wc": np.ascontiguousarray(Wc, dtype=np.float32),
                "bc": np.ascontiguousarray(bc.reshape(1, -1), dtype=np.float32),
            }
        )

    res = run_bass_kernel_spmd(
        nc, in_maps, core_ids=list(range(n_cores)), trace=trace
    )
    shards = [res.results[p]["out"][pos[p]] for p in range(n_cores)]
    return np.concatenate(shards, axis=0), res


def kernel(feat, state, src, dst, edge_weight, Wzr, bzr, Wc, bc):
    out, _ = run(feat, state, src, dst, edge_weight, Wzr, bzr, Wc, bc, trace=False)
    return out
